# revision 3
# baseline (speedup 1.0000x reference)
"""Trainium2 Bass kernel for nn_Bspline_19335942766607.

inputs [16, 25, 2048] f32 -> flow [16, 25, 192, 192, 2] f32.

Math: each of the 400 samples is a 32x32x2 control-point grid, bilinearly
resampled to 192x192 per channel and scaled by -192.  The query grid is
fixed, so per sample and channel this is two constant-matrix products:
    T_c = (-192 * Ay) @ P_c        Ay [192,32] interpolation matrix
    D_c = T_c @ Ax^T               Ax [192,32]

Kernel design (per core, 50 samples; pure data-parallel over 8 cores):
- single-fp16 arithmetic with fp32 PSUM accumulation: control points are
  host-cast to fp16, tt = 3*tt16 with exact fp16 constants (-192*Ay
  entries are integers; 3*Ax entries are k/64).  End-to-end rel err
  ~1e-3 (fp16 rounding of p, tt, and the output), far inside the 2e-2
  gate, at half the matmul / twice the copy throughput of a split
  scheme.
- samples processed in PAIRS: tt PSUM tile [64, 384] (cols = sample a
  0:192 | sample b 192:384), one K=32 matmul per sample.  ACT converts
  to hl [64, 384] fp16 (scale 1/3).
- stage-2: ONE matmul per stripe k: lhsT = hl[:, k:384:3] [64, 128]
  (stride 3 runs through sample a's 192 cols straight into sample b's,
  since 3*64 = 192), rhs = axt3[0:64] [64, 384].  Output stripe k holds
  pair-row r = 3p + k on partition p, so the three stripes concatenate
  into one fully-contiguous [128 x 2304 B] fp16 block per pair.
- OUTPUT IS WRITTEN fp16 (host upcasts to f32 after gather): halves the
  HBM write traffic, which is the roofline term.  Two pairs share one
  SBUF tile and leave in a single ~0.6 MB contiguous DMA, round-robined
  across the sync / gpsimd / scalar DGE rings.
- PE cost: 2x192 + 3x384 = 1536 cycles/pair; ACT: fp16-convert + one
  stripe copy; DVE: two stripe copies.  All engines land ~13-20 us,
  the output DMA ~15-20 us.
- PSUM: tt triple-buffered + 5 shared stripe slots (8 banks).
- emission is software-pipelined (stage-1 of pair j+2 and convert of
  pair j+1 are emitted between stage-2 and copies of pair j).
"""

import sys

if "/opt/trn_rl_repo" not in sys.path:
    sys.path.insert(0, "/opt/trn_rl_repo")

import numpy as np

import concourse.mybir as mybir
from concourse import bacc
from concourse.bass import ds
from concourse.bass_utils import run_bass_kernel_spmd
from concourse.tile import TileContext

F32 = mybir.dt.float32
F16 = mybir.dt.float16

B, T = 16, 25
H, W = 192, 192
G = 32
N_CORES = 8
N_SAMPLES = B * T                   # 400
S_PER_CORE = N_SAMPLES // N_CORES   # 50
FW = 2 * W                          # 384
H2 = 2 * H                          # 384 (pair cols: sample a | sample b)


def _interp_weights(size_out, size_in):
    q = (np.arange(size_out, dtype=np.float32) / np.float32(size_out)) * np.float32(
        size_in - 1
    )
    f = np.clip(np.floor(q), np.float32(0.0), np.float32(size_in - 2))
    idx0 = f.astype(np.int32)
    alpha = np.clip(q - f, np.float32(0.0), np.float32(1.0))
    return idx0, alpha


def _make_constants():
    """ayt16 [32,192] = fp16((-192*Ay)^T); axt3 [64,384] = fp16(3*Ax)^T
    channel-interleaved."""
    y0, ay = _interp_weights(H, G)
    x0, ax = _interp_weights(W, G)
    Ay = np.zeros((H, G), dtype=np.float32)
    Ay[np.arange(H), y0] = np.float32(1.0) - ay
    Ay[np.arange(H), y0 + 1] += ay
    Ax = np.zeros((W, G), dtype=np.float32)
    Ax[np.arange(W), x0] = np.float32(1.0) - ax
    Ax[np.arange(W), x0 + 1] += ax
    ayt16 = (np.float32(-H) * Ay).T.astype(np.float16)        # [32, 192]
    ax3 = (np.float32(3.0) * Ax).T.astype(np.float16)         # [32, 192]
    axt3 = np.zeros((2 * G, FW), dtype=np.float16)
    for c in range(2):
        axt3[c * G : (c + 1) * G, c::2] = ax3
    return np.ascontiguousarray(ayt16), np.ascontiguousarray(axt3)


def build(n_samples=S_PER_CORE, n_reps=1):
    """Per-core Bass program (SPMD across 8 cores)."""
    assert n_samples % 2 == 0
    npair = n_samples // 2
    nc = bacc.Bacc(None, target_bir_lowering=False, debug=False)
    # p16 arrives host-transposed [G, n*64] so the load is one contiguous DMA
    p_ext = nc.declare_dram_parameter("p16", [G, n_samples * 2 * G], F16, isOutput=False)
    ayt_ext = nc.declare_dram_parameter("ayt16", [G, H], F16, isOutput=False)
    axt_ext = nc.declare_dram_parameter("axt3", [2 * G, FW], F16, isOutput=False)
    out_ext = nc.declare_dram_parameter(
        "out", [n_samples, H, FW], F16, isOutput=True
    )
    dma_batch = 2

    with TileContext(nc) as tc:
        with (
            tc.tile_pool(name="const", bufs=1) as cpool,
            tc.tile_pool(name="work", bufs=4) as wpool,
            tc.tile_pool(name="psum", bufs=1, space="PSUM") as pspool,
        ):
            ayt_sb = cpool.tile([G, H], F16)
            nc.sync.dma_start(out=ayt_sb[:], in_=ayt_ext[:])
            axt_sb = cpool.tile([2 * G, FW], F16)
            nc.sync.dma_start(out=axt_sb[:], in_=axt_ext[:])
            p_sb = cpool.tile([G, n_samples * 2 * G], F16)
            nc.sync.dma_start(out=p_sb[:], in_=p_ext[:])

            dma_cycle = [nc.sync, nc.gpsimd, nc.scalar]

            for _rep in range(n_reps):

                def s1(j):
                    # one K=32 matmul per sample; sample s -> cols s*192:+192
                    tt_ps = pspool.tile([2 * G, H2], F32, tag="tt", bufs=3, name="tt_ps")
                    for s in (0, 1):
                        i = 2 * j + s
                        nc.tensor.matmul(
                            tt_ps[:, s * H : (s + 1) * H],
                            p_sb[:, ds(i * 2 * G, 2 * G)],
                            ayt_sb[:],
                            start=True, stop=True, tile_position=(0, 0),
                        )
                    return tt_ps

                def ctt(tt_ps):
                    # hl = fp16(tt/3) on ACT
                    hl = wpool.tile([2 * G, H2], F16, tag="hl")
                    nc.scalar.activation(
                        hl[:], tt_ps[:],
                        mybir.ActivationFunctionType.Copy, scale=1.0 / 3.0,
                    )
                    return hl

                def s2(hl):
                    # stripe k holds pair-output rows r = 3p + k: p < 64 ->
                    # sample a row 3p+k (hl col k+3p < 192), p >= 64 ->
                    # sample b row 3(p-64)+k (col k+3p >= 192).
                    ps = []
                    for k in range(3):
                        pk = pspool.tile([128, FW], F32, tag="pk", bufs=5, name="pk")
                        nc.tensor.matmul(
                            pk[:], hl[:, k : H2 : 3], axt_sb[:],
                            start=True, stop=True, tile_position=(0, 0),
                        )
                        ps.append(pk)
                    return ps

                o_sb_cur = [None]

                def emit_out(j, psums):
                    bi = j % dma_batch
                    if bi == 0:
                        o_sb_cur[0] = wpool.tile(
                            [128, dma_batch * 3 * FW], F16, tag="o_sb", name="o_sb"
                        )
                    o_sb = o_sb_cur[0]
                    off = bi * 3 * FW
                    for k in range(3):
                        dst = o_sb[:, off + k * FW : off + (k + 1) * FW]
                        if k == 1:
                            nc.scalar.copy(out=dst, in_=psums[k][:])
                        else:
                            nc.vector.tensor_copy(out=dst, in_=psums[k][:])
                    if bi == dma_batch - 1 or j == npair - 1:
                        nb = bi + 1
                        s = 2 * (j - bi)
                        eng = dma_cycle[(j // dma_batch) % len(dma_cycle)]
                        # DRAM row (384*jj + 3p + k) <- o_sb[p, jj*1152+k*384+wc]
                        dst = (
                            out_ext[s : s + 2 * nb]
                            .rearrange("s h f -> (s h) f")
                            .rearrange("(jj p k) f -> p jj k f", p=128, k=3)
                            .rearrange("p jj k f -> p jj (k f)")
                        )
                        src = o_sb[:, 0 : nb * 3 * FW].rearrange(
                            "p (jj kf) -> p jj kf", jj=nb
                        )
                        eng.dma_start(out=dst, in_=src)

                tt_ps_q = {0: s1(0)}
                tt_sb_q = {0: ctt(tt_ps_q.pop(0))}
                if npair > 1:
                    tt_ps_q[1] = s1(1)
                for j in range(npair):
                    psums = s2(tt_sb_q.pop(j))
                    if j + 1 < npair:
                        tt_sb_q[j + 1] = ctt(tt_ps_q.pop(j + 1))
                    if j + 2 < npair:
                        tt_ps_q[j + 2] = s1(j + 2)
                    emit_out(j, psums)
    nc.finalize()
    return nc


_CACHE = {}


def _get_nc(n_reps=1):
    if n_reps not in _CACHE:
        _CACHE[n_reps] = build(n_reps=n_reps)
    return _CACHE[n_reps]


def prep_inputs(p_full):
    """p_full [400, 32, 64] f32 (raw [g, (g',c)]) -> per-core in_maps."""
    ayt16, axt3 = _make_constants()
    # deinterleave channels: column m = c*32 + g'
    p_d = (
        p_full.reshape(N_SAMPLES, G, G, 2)
        .transpose(0, 1, 3, 2)
        .reshape(N_SAMPLES, G, 2 * G)
    )
    p16 = p_d.astype(np.float16)
    # host transpose to [core, G, 50*64] (partition-major, contiguous load)
    p16_t = np.ascontiguousarray(
        p16.reshape(N_CORES, S_PER_CORE, G, 2 * G)
        .transpose(0, 2, 1, 3)
        .reshape(N_CORES, G, S_PER_CORE * 2 * G)
    )
    return [
        {"p16": p16_t[c], "ayt16": ayt16, "axt3": axt3}
        for c in range(N_CORES)
    ]


def run_on_hw(p_full, n_reps=1):
    """p_full [400, 32, 64] f32 -> out [400, 192, 384] f32."""
    in_maps = prep_inputs(p_full)
    nc = _get_nc(n_reps)
    res = run_bass_kernel_spmd(nc, in_maps, list(range(N_CORES))).results
    out = np.stack([res[c]["out"] for c in range(N_CORES)])
    return out.reshape(N_SAMPLES, H, FW).astype(np.float32)


def kernel(inputs):
    inputs = np.ascontiguousarray(np.asarray(inputs), dtype=np.float32)
    assert inputs.shape == (B, T, 2 * G * G), inputs.shape
    out = run_on_hw(inputs.reshape(N_SAMPLES, G, 2 * G))
    return out.reshape(B, T, H, W, 2)


# revision 7
# speedup vs baseline: 1.2200x; 1.2200x over previous
"""Trainium2 Bass kernel for nn_Bspline_19335942766607.

inputs [16, 25, 2048] f32 -> flow [16, 25, 192, 192, 2] f32.

Math: each of the 400 samples is a 32x32x2 control-point grid, bilinearly
resampled to 192x192 per channel and scaled by -192.  The query grid is
fixed, so per sample and channel this is two constant-matrix products:
    T_c = (-192 * Ay) @ P_c        Ay [192,32] interpolation matrix
    D_c = T_c @ Ax^T               Ax [192,32]

Kernel design (per core, 50 samples; pure data-parallel over 8 cores):
- single-fp16 arithmetic with fp32 PSUM accumulation: control points are
  host-cast to fp16, tt = 3*tt16 with exact fp16 constants (-192*Ay
  entries are integers; 3*Ax entries are k/64).  End-to-end rel err
  ~1e-3 (fp16 rounding of p, tt, and the output), far inside the 2e-2
  gate, at half the matmul / twice the copy throughput of a split
  scheme.
- samples processed in PAIRS: tt PSUM tile [64, 384] (cols = sample a
  0:192 | sample b 192:384), one K=32 matmul per sample.  ACT converts
  to hl [64, 384] fp16 (scale 1/3).
- stage-2: ONE matmul per stripe k: lhsT = hl[:, k:384:3] [64, 128]
  (stride 3 runs through sample a's 192 cols straight into sample b's,
  since 3*64 = 192), rhs = axt3[0:64] [64, 384].  Output stripe k holds
  pair-row r = 3p + k on partition p, so the three stripes concatenate
  into one fully-contiguous [128 x 2304 B] fp16 block per pair.
- OUTPUT IS WRITTEN fp16 (host upcasts to f32 after gather): halves the
  HBM write traffic, which is the roofline term.  Two pairs share one
  SBUF tile and leave in a single ~0.6 MB contiguous DMA, round-robined
  across the sync / gpsimd / scalar DGE rings.
- PE cost: 2x192 + 3x384 = 1536 cycles/pair; ACT: fp16-convert + one
  stripe copy; DVE: two stripe copies.  All engines land ~13-20 us,
  the output DMA ~15-20 us.
- PSUM: tt triple-buffered + 5 shared stripe slots (8 banks).
- emission is software-pipelined (stage-1 of pair j+2 and convert of
  pair j+1 are emitted between stage-2 and copies of pair j).
"""

import sys

if "/opt/trn_rl_repo" not in sys.path:
    sys.path.insert(0, "/opt/trn_rl_repo")

import numpy as np

import concourse.mybir as mybir
from concourse import bacc
from concourse.bass import ds
from concourse.bass_utils import run_bass_kernel_spmd
from concourse.tile import TileContext

F32 = mybir.dt.float32
F16 = mybir.dt.float16

B, T = 16, 25
H, W = 192, 192
G = 32
N_CORES = 8
N_SAMPLES = B * T                   # 400
S_PER_CORE = N_SAMPLES // N_CORES   # 50
FW = 2 * W                          # 384
H2 = 2 * H                          # 384 (pair cols: sample a | sample b)


def _interp_weights(size_out, size_in):
    q = (np.arange(size_out, dtype=np.float32) / np.float32(size_out)) * np.float32(
        size_in - 1
    )
    f = np.clip(np.floor(q), np.float32(0.0), np.float32(size_in - 2))
    idx0 = f.astype(np.int32)
    alpha = np.clip(q - f, np.float32(0.0), np.float32(1.0))
    return idx0, alpha


def _make_constants():
    """ayt16 [32,192] = fp16((-192*Ay)^T); axt3 [64,384] = fp16(3*Ax)^T
    channel-interleaved."""
    y0, ay = _interp_weights(H, G)
    x0, ax = _interp_weights(W, G)
    Ay = np.zeros((H, G), dtype=np.float32)
    Ay[np.arange(H), y0] = np.float32(1.0) - ay
    Ay[np.arange(H), y0 + 1] += ay
    Ax = np.zeros((W, G), dtype=np.float32)
    Ax[np.arange(W), x0] = np.float32(1.0) - ax
    Ax[np.arange(W), x0 + 1] += ax
    ayt16 = (np.float32(-H) * Ay).T.astype(np.float16)        # [32, 192]
    ax3 = (np.float32(3.0) * Ax).T.astype(np.float16)         # [32, 192]
    axt3 = np.zeros((2 * G, FW), dtype=np.float16)
    for c in range(2):
        axt3[c * G : (c + 1) * G, c::2] = ax3
    return np.ascontiguousarray(ayt16), np.ascontiguousarray(axt3)


def build(n_samples=S_PER_CORE, n_reps=1, n_loop=1):
    """Per-core Bass program (SPMD across 8 cores).

    n_reps: python-unrolled repetitions of the whole workload (timing).
    n_loop: hardware-loop (tc.For_i) iterations around those reps --
        keeps the instruction count flat for large timing contrasts.
    """
    assert n_samples % 2 == 0
    npair = n_samples // 2
    nc = bacc.Bacc(None, target_bir_lowering=False, debug=False)
    # p16 arrives host-transposed [G, n*64] so the load is one contiguous DMA
    p_ext = nc.declare_dram_parameter("p16", [G, n_samples * 2 * G], F16, isOutput=False)
    ayt_ext = nc.declare_dram_parameter("ayt16", [G, H], F16, isOutput=False)
    axt_ext = nc.declare_dram_parameter("axt3", [2 * G, FW], F16, isOutput=False)
    out_ext = nc.declare_dram_parameter(
        "out", [n_samples, H, FW], F16, isOutput=True
    )
    dma_batch = 2

    with TileContext(nc) as tc:
        with (
            tc.tile_pool(name="const", bufs=1) as cpool,
            tc.tile_pool(name="work", bufs=4) as wpool,
            tc.tile_pool(name="psum", bufs=1, space="PSUM") as pspool,
        ):
            ayt_sb = cpool.tile([G, H], F16)
            nc.sync.dma_start(out=ayt_sb[:], in_=ayt_ext[:])
            axt_sb = cpool.tile([2 * G, FW], F16)
            nc.sync.dma_start(out=axt_sb[:], in_=axt_ext[:])
            p_sb = cpool.tile([G, n_samples * 2 * G], F16)
            nc.sync.dma_start(out=p_sb[:], in_=p_ext[:])

            dma_cycle = [nc.sync, nc.gpsimd, nc.scalar]

            def rep_body():

                def s1(j):
                    # one K=32 matmul per sample; sample s -> cols s*192:+192
                    tt_ps = pspool.tile([2 * G, H2], F32, tag="tt", bufs=3, name="tt_ps")
                    for s in (0, 1):
                        i = 2 * j + s
                        nc.tensor.matmul(
                            tt_ps[:, s * H : (s + 1) * H],
                            p_sb[:, ds(i * 2 * G, 2 * G)],
                            ayt_sb[:],
                            start=True, stop=True, tile_position=(0, 0),
                        )
                    return tt_ps

                def ctt(tt_ps):
                    # hl = fp16(tt/3) on ACT
                    hl = wpool.tile([2 * G, H2], F16, tag="hl")
                    nc.scalar.activation(
                        hl[:], tt_ps[:],
                        mybir.ActivationFunctionType.Copy, scale=1.0 / 3.0,
                    )
                    return hl

                def s2(hl):
                    # stripe k holds pair-output rows r = 3p + k: p < 64 ->
                    # sample a row 3p+k (hl col k+3p < 192), p >= 64 ->
                    # sample b row 3(p-64)+k (col k+3p >= 192).
                    ps = []
                    for k in range(3):
                        pk = pspool.tile([128, FW], F32, tag="pk", bufs=5, name="pk")
                        nc.tensor.matmul(
                            pk[:], hl[:, k : H2 : 3], axt_sb[:],
                            start=True, stop=True, tile_position=(0, 0),
                        )
                        ps.append(pk)
                    return ps

                o_sb_cur = [None]

                def emit_out(j, psums):
                    bi = j % dma_batch
                    if bi == 0:
                        o_sb_cur[0] = wpool.tile(
                            [128, dma_batch * 3 * FW], F16, tag="o_sb", name="o_sb"
                        )
                    o_sb = o_sb_cur[0]
                    off = bi * 3 * FW
                    for k in range(3):
                        dst = o_sb[:, off + k * FW : off + (k + 1) * FW]
                        if k == 1:
                            nc.scalar.copy(out=dst, in_=psums[k][:])
                        else:
                            nc.vector.tensor_copy(out=dst, in_=psums[k][:])
                    if bi == dma_batch - 1 or j == npair - 1:
                        nb = bi + 1
                        s = 2 * (j - bi)
                        eng = dma_cycle[(j // dma_batch) % len(dma_cycle)]
                        # DRAM row (384*jj + 3p + k) <- o_sb[p, jj*1152+k*384+wc]
                        dst = (
                            out_ext[s : s + 2 * nb]
                            .rearrange("s h f -> (s h) f")
                            .rearrange("(jj p k) f -> p jj k f", p=128, k=3)
                            .rearrange("p jj k f -> p jj (k f)")
                        )
                        src = o_sb[:, 0 : nb * 3 * FW].rearrange(
                            "p (jj kf) -> p jj kf", jj=nb
                        )
                        eng.dma_start(out=dst, in_=src)

                tt_ps_q = {0: s1(0)}
                tt_sb_q = {0: ctt(tt_ps_q.pop(0))}
                if npair > 1:
                    tt_ps_q[1] = s1(1)
                for j in range(npair):
                    psums = s2(tt_sb_q.pop(j))
                    if j + 1 < npair:
                        tt_sb_q[j + 1] = ctt(tt_ps_q.pop(j + 1))
                    if j + 2 < npair:
                        tt_ps_q[j + 2] = s1(j + 2)
                    emit_out(j, psums)

            if n_loop == 1:
                for _rep in range(n_reps):
                    rep_body()
            else:
                with tc.For_i(0, n_loop, 1):
                    for _rep in range(n_reps):
                        rep_body()
    nc.finalize()
    return nc


_CACHE = {}


def _get_nc(n_reps=1, n_loop=1):
    key = (n_reps, n_loop)
    if key not in _CACHE:
        _CACHE[key] = build(n_reps=n_reps, n_loop=n_loop)
    return _CACHE[key]


def prep_inputs(p_full):
    """p_full [400, 32, 64] f32 (raw [g, (g',c)]) -> per-core in_maps."""
    ayt16, axt3 = _make_constants()
    # deinterleave channels: column m = c*32 + g'
    p_d = (
        p_full.reshape(N_SAMPLES, G, G, 2)
        .transpose(0, 1, 3, 2)
        .reshape(N_SAMPLES, G, 2 * G)
    )
    p16 = p_d.astype(np.float16)
    # host transpose to [core, G, 50*64] (partition-major, contiguous load)
    p16_t = np.ascontiguousarray(
        p16.reshape(N_CORES, S_PER_CORE, G, 2 * G)
        .transpose(0, 2, 1, 3)
        .reshape(N_CORES, G, S_PER_CORE * 2 * G)
    )
    return [
        {"p16": p16_t[c], "ayt16": ayt16, "axt3": axt3}
        for c in range(N_CORES)
    ]


def run_on_hw(p_full, n_reps=1):
    """p_full [400, 32, 64] f32 -> out [400, 192, 384] f32."""
    in_maps = prep_inputs(p_full)
    nc = _get_nc(n_reps)
    res = run_bass_kernel_spmd(nc, in_maps, list(range(N_CORES))).results
    out = np.stack([res[c]["out"] for c in range(N_CORES)])
    return out.reshape(N_SAMPLES, H, FW).astype(np.float32)


def kernel(inputs):
    inputs = np.ascontiguousarray(np.asarray(inputs), dtype=np.float32)
    assert inputs.shape == (B, T, 2 * G * G), inputs.shape
    out = run_on_hw(inputs.reshape(N_SAMPLES, G, 2 * G))
    return out.reshape(B, T, H, W, 2)
